# revision 5
# baseline (speedup 1.0000x reference)
"""CLIP text embeddings (token-id gather + position-embedding add) on 8
Trainium2 NeuronCores via a raw Bass kernel.

Sharding: the 768-wide d_model axis is split 8 ways (96 columns per
core); the 77 token ids are replicated. Each core holds the full
49408-row token table restricted to its 96 columns, so every output
element is owned by exactly one core and no collective is needed (a
Megatron-style vocab-parallel split would pay a 236 KB AllReduce that
costs more than this whole kernel).

Per-core device program (3 serialized DMA stages, 2 engines):

  1. sync engine (HWDGE): one DMA loads a packed [128, 97] f32 input:
     columns 0..95 are the position-embedding slice (rows 0..76),
     column 96 carries the token ids bitcast to f32, one id per SBUF
     partition (an id-only [128,1] DMA costs an extra ~0.7 us, and a
     row-major id layout is rejected by the DGE).
  2. gpsimd: one indirect DMA gathers the 77 requested token rows from
     HBM and ADDS them onto the accumulator in flight (CCE
     compute_op=add) -- offsets read from column 96 via bitcast. This
     fuses the position add; no vector-engine pass exists at all.
  3. sync engine: one DMA stores the accumulator [77, 96] to HBM.

The host concatenates the 8 column shards into [1, 77, 768].

Raw Bass (no TileContext) keeps the kernel tail free of Tile's
drain + EVSEM barrier (~9-17 us). Measured steady-state device time
per invocation: ~7.2 us (serialized unrolled-loop delta method).
"""

import sys

sys.path.insert(0, "/opt/trn_rl_repo")

import numpy as np

VOCAB = 49408
D_MODEL = 768
SEQ_LEN = 77
N_CORES = 8
D_SHARD = D_MODEL // N_CORES  # 96
P = 128  # SBUF partitions

_cache = {}


def _build_nc():
    from concourse import bass, mybir

    D = D_SHARD
    nc = bass.Bass(num_devices=N_CORES)
    inp_d = nc.dram_tensor(
        "inp", [P, D + 1], mybir.dt.float32, kind="ExternalInput"
    )
    tok_d = nc.dram_tensor(
        "tok_w", [VOCAB, D], mybir.dt.float32, kind="ExternalInput"
    )
    out_d = nc.dram_tensor(
        "out", [SEQ_LEN, D], mybir.dt.float32, kind="ExternalOutput"
    )

    with (
        nc.sbuf_tensor([P, D + 1], mybir.dt.float32) as acc_t,
        nc.semaphore() as s_in,
        nc.semaphore() as s_tok,
        nc.semaphore() as s_out,
        nc.Block() as block,
    ):

        @block.sync
        def _(sync):
            sync.dma_start(out=acc_t[:], in_=inp_d[:]).then_inc(s_in, 16)
            sync.wait_ge(s_tok, 16)
            sync.dma_start(out=out_d[:], in_=acc_t[:SEQ_LEN, :D]).then_inc(
                s_out, 16
            )
            sync.wait_ge(s_out, 16)

        @block.gpsimd
        def _(gp):
            gp.wait_ge(s_in, 16)
            nc.gpsimd.indirect_dma_start(
                out=acc_t[:SEQ_LEN, :D],
                out_offset=None,
                in_=tok_d[:],
                in_offset=bass.IndirectOffsetOnAxis(
                    ap=acc_t[:SEQ_LEN, D : D + 1].bitcast(mybir.dt.int32),
                    axis=0,
                ),
                compute_op=mybir.AluOpType.add,
            ).then_inc(s_tok, 16)

    return nc


def _get_runner():
    """Build the Bass module once and cache a jitted SPMD callable so
    repeat kernel() calls skip retracing/compilation."""
    if "runner" in _cache:
        return _cache["runner"]

    import jax
    from jax.sharding import Mesh, PartitionSpec
    from jax.experimental.shard_map import shard_map
    from concourse import bass2jax, mybir

    bass2jax.install_neuronx_cc_hook()
    nc = _build_nc()
    partition_name = nc.partition_id_tensor.name if nc.partition_id_tensor else None

    in_names, out_names, out_avals, zero_outs = [], [], [], []
    for alloc in nc.m.functions[0].allocations:
        if not isinstance(alloc, mybir.MemoryLocationSet):
            continue
        name = alloc.memorylocations[0].name
        if alloc.kind == "ExternalInput":
            if name != partition_name:
                in_names.append(name)
        elif alloc.kind == "ExternalOutput":
            out_names.append(name)
            shape = tuple(alloc.tensor_shape)
            dtype = mybir.dt.np(alloc.dtype)
            out_avals.append(jax.core.ShapedArray(shape, dtype))
            zero_outs.append(np.zeros(shape, dtype))

    n_params = len(in_names)
    n_outs = len(out_avals)
    all_in_names = list(in_names) + list(out_names)
    if partition_name is not None:
        all_in_names.append(partition_name)

    def _body(*args):
        operands = list(args)
        if partition_name is not None:
            operands.append(bass2jax.partition_id_tensor())
        outs = bass2jax._bass_exec_p.bind(
            *operands,
            out_avals=tuple(out_avals),
            in_names=tuple(all_in_names),
            out_names=tuple(out_names),
            lowering_input_output_aliases=(),
            sim_require_finite=True,
            sim_require_nnan=True,
            nc=nc,
        )
        return tuple(outs)

    devices = jax.devices()[:N_CORES]
    assert len(devices) == N_CORES, f"need {N_CORES} devices, got {len(devices)}"
    mesh = Mesh(np.asarray(devices), ("core",))
    sharded = jax.jit(
        shard_map(
            _body,
            mesh=mesh,
            in_specs=(PartitionSpec("core"),) * (n_params + n_outs),
            out_specs=(PartitionSpec("core"),) * n_outs,
            check_rep=False,
        ),
        donate_argnums=tuple(range(n_params, n_params + n_outs)),
        keep_unused=True,
    )

    def run(in_maps):
        concat_in = [
            np.concatenate([np.asarray(m[name]) for m in in_maps], axis=0)
            for name in in_names
        ]
        concat_zeros = [
            np.zeros((N_CORES * z.shape[0], *z.shape[1:]), z.dtype)
            for z in zero_outs
        ]
        out_arrs = sharded(*concat_in, *concat_zeros)
        return [
            {
                name: np.asarray(out_arrs[i]).reshape(
                    N_CORES, *out_avals[i].shape
                )[c]
                for i, name in enumerate(out_names)
            }
            for c in range(N_CORES)
        ]

    _cache["runner"] = run
    return run


def kernel(
    input_ids: np.ndarray,
    position_ids: np.ndarray,
    token_embedding_weight: np.ndarray,
    position_embedding_weight: np.ndarray,
) -> np.ndarray:
    run = _get_runner()

    ids = np.asarray(input_ids).astype(np.int32, copy=False)
    assert ids.shape == (SEQ_LEN,), ids.shape
    tok = np.asarray(token_embedding_weight, dtype=np.float32)
    pos_table = np.asarray(position_embedding_weight, dtype=np.float32)
    pids = np.asarray(position_ids).astype(np.int64, copy=False)
    if np.array_equal(pids, np.arange(SEQ_LEN)):
        pos = pos_table
    else:
        # CLIP always uses arange positions; reorder the tiny replicated
        # table during input prep if a caller ever passes something else.
        pos = pos_table[pids]

    ids_f32 = ids.view(np.float32)
    in_maps = []
    for c in range(N_CORES):
        sl = slice(c * D_SHARD, (c + 1) * D_SHARD)
        packed = np.zeros((P, D_SHARD + 1), np.float32)
        packed[:SEQ_LEN, :D_SHARD] = pos[:, sl]
        packed[:SEQ_LEN, D_SHARD] = ids_f32
        in_maps.append(
            {
                "inp": packed,
                "tok_w": np.ascontiguousarray(tok[:, sl]),
            }
        )

    results = run(in_maps)
    out = np.concatenate([results[c]["out"] for c in range(N_CORES)], axis=1)
    return out[None, :, :]
